# revision 12
# baseline (speedup 1.0000x reference)
"""CBOW negative-sampling loss on 8 Trainium2 NeuronCores.

Reference computation:
    v      = V_emb[ctx] * mask_v                  # [B,1,E]
    u      = U_emb[tgt] * mask_u                  # [B,1,E]
    u_neg  = -(U_emb[neg] * mask_neg)             # [B,K,E]
    pos    = <u, v>
    neg    = sum_k <u_neg_k, v>
    loss   = -mean(log_sigmoid(pos) + log_sigmoid(neg))
           = mean(softplus(-pos) + softplus(negsum)),  negsum = -neg

Strategy: data-parallel over B across 8 cores, tables replicated.  Both
embedding tables are concatenated host-side into one [2*VOCAB, E] table
(cast to bf16) so each batch row needs 12 row-gathers (ctx, tgt, 10 neg)
from a single table.  The only high-throughput gather on TRN2 is the
GPSIMD dma_gather (mlp library), whose indices are int16 (< 32768) —
too small for 200k rows.  So gathers are two-level, per "super group"
(sg) of 8 row-tiles = 12288 gather positions:

  L1: the sg's gather ids are sorted and split into 7 static-capacity
      range buckets; per bucket one compact dma_gather call (id - base
      fits int16) pulls the rows into an SBUF tile in sorted order.
  stage: one contiguous 128-descriptor DMA copies that tile to a DRAM
      scratch laid out so sorted rank r lives at scratch row
      r = partition*TCOLS + column (per-partition contiguous).
  L2: one positional dma_gather (indices = staging rank < 14080, int16)
      scatters all 12288 rows into batch-position layout [p, col*E:..].

Masks stream into an identically-laid-out tile; the DVE multiplies
X*M, multiplies by the broadcast masked-ctx row (vm), and the scalar
engine accumulates per-tile pos/neg dot sums (ACT Copy + accum_out).
Final softplus tail in f32; per-core [128,1] partial sums are reduced
on the host.  bf16 storage throughout the bulk path: the final value
is a mean over 65536 rows, so per-row rounding noise averages out
(measured ~1e-4 relative).
"""

import numpy as np

B, K, VOCAB, E = 65536, 10, 100000, 128
NCORES = 8
P = 128
SLOTS = 2 + K

# two-level gather geometry
SG_TILES = 8                       # row-tiles per super group
SG_POS = SG_TILES * P * SLOTS      # gather positions per sg = 12288
BUCKET = 32768
# static per-bucket capacities (rows, multiples of 128) sized for uniform
# ids: slot0 uniform [0,100000), slots1-11 uniform [100000,200000)
CAPS = (512, 512, 512, 3840, 4096, 4096, 512)
TROWS = sum(CAPS)                  # 14080 staging rows per sg
TCOLS = TROWS // P                 # 110

_prog_cache = {}

# debug ablation: 1=L1 only, 2=+stage, 3=+L2, 4=+masks, 5=+compute, 6=full
ABLATE = 6
# number of SWDGE queues (GPSIMD core pairs) to spread gathers over, 1-4
N_QUEUES = 1
# pad L1 idx lists with trailing -1 (trimmed by ucode) instead of 0
PAD_NEG1 = False
# split the L2 gather into this many chunks (round-robin over queues)
L2_SPLIT = 1


def _bf16():
    import ml_dtypes

    return np.dtype(ml_dtypes.bfloat16)


def _build_program(bsh, vocab, k, e, sg_tiles, caps, ncores):
    import concourse.bacc as bacc
    import concourse.tile as tile
    from concourse import library_config, mybir

    f32 = mybir.dt.float32
    bf16 = mybir.dt.bfloat16
    i16 = mybir.dt.int16
    slots = 2 + k
    tiles = bsh // P
    assert tiles % sg_tiles == 0
    nsg = tiles // sg_tiles
    sg_pos = sg_tiles * P * slots
    trows = sum(caps)
    tcols = trows // P
    xcols = sg_tiles * slots           # X columns per sg
    l1_cols = trows // 16              # int16 idx columns per sg (16-wrap)
    l2_cols = sg_pos // 16

    nc = bacc.Bacc(
        "TRN2", target_bir_lowering=False, debug=False, num_devices=ncores,
        num_swdge_queues=N_QUEUES,
    )
    w = nc.dram_tensor("w", [2 * vocab, e], bf16, kind="ExternalInput")
    l1i = nc.dram_tensor("l1i", [P, nsg * l1_cols], i16, kind="ExternalInput")
    l2i = nc.dram_tensor("l2i", [P, nsg * l2_cols], i16, kind="ExternalInput")
    mv = nc.dram_tensor("mv", [bsh, e], bf16, kind="ExternalInput")
    mu = nc.dram_tensor("mu", [bsh, e], bf16, kind="ExternalInput")
    mn = nc.dram_tensor("mn", [bsh, k * e], bf16, kind="ExternalInput")
    out = nc.dram_tensor("out", [P, 1], f32, kind="ExternalOutput")

    mult = mybir.AluOpType.mult
    add = mybir.AluOpType.add
    AF = mybir.ActivationFunctionType

    with tile.TileContext(nc) as tc:
        with (
            tc.tile_pool(name="sb", bufs=2) as pool,
            tc.tile_pool(name="stg", bufs=2, space="DRAM") as dpool,
            tc.tile_pool(name="acc", bufs=1) as apool,
        ):
            nc.gpsimd.load_library(library_config.mlp)

            post = apool.tile([P, tiles], f32, tag="post")
            negt = apool.tile([P, tiles], f32, tag="negt")

            for sg in range(nsg):
                # --- index tiles for this sg ---
                l1t = pool.tile([P, l1_cols], i16, tag="l1t")
                nc.sync.dma_start(
                    out=l1t[:], in_=l1i[:, sg * l1_cols : (sg + 1) * l1_cols]
                )
                l2t = pool.tile([P, l2_cols], i16, tag="l2t")
                nc.sync.dma_start(
                    out=l2t[:], in_=l2i[:, sg * l2_cols : (sg + 1) * l2_cols]
                )

                # --- L1: bucketed compact gathers into sorted-order tile ---
                # Buckets are spread across the 4 SWDGE queues (each queue =
                # its own GPSIMD core pair generating descriptors), so the
                # per-bucket generation runs concurrently.  Trailing -1
                # padding in the idx lists is trimmed by the ucode before
                # descriptor generation, so padding costs nothing.
                T = pool.tile([P, tcols * e], bf16, tag="T")
                colbase = 0
                # big buckets 3,4,5 get their own queue; small ones share q3
                bq = {0: 3, 1: 3, 2: 3, 3: 0, 4: 1, 5: 2, 6: 3}
                for bi, cap in enumerate(caps):
                    nc.gpsimd.dma_gather(
                        out_ap=T[:, colbase * e : (colbase + cap // P) * e]
                        .rearrange("p (c e) -> p c e", e=e),
                        in_ap=w[bi * BUCKET :, :],
                        idxs_ap=l1t[:, colbase * 8 : (colbase + cap // P) * 8],
                        num_idxs=cap,
                        num_idxs_reg=cap,
                        elem_size=e,
                        single_packet=False,
                        queue_num=bq[bi] % N_QUEUES,
                    )
                    colbase += cap // P

                if ABLATE < 2:
                    continue
                # --- stage to DRAM: sorted rank r = p*tcols + c ---
                stg = dpool.tile([trows, e], bf16, tag="stg")
                nc.sync.dma_start(
                    out=stg[:]
                    .rearrange("(p c) e -> p c e", p=P)
                    .rearrange("p c e -> p (c e)"),
                    in_=T[:],
                )

                if ABLATE < 3:
                    continue
                # --- L2: positional gather into batch layout ---
                # split into 4 chunks, one per SWDGE queue, for concurrent
                # descriptor generation
                X = pool.tile([P, xcols * e], bf16, tag="X")
                n_chunk = L2_SPLIT
                chunk = sg_pos // n_chunk          # 3072 positions
                ccols = chunk // P                 # 24 output cols of e
                icols = chunk // 16                # 192 idx cols
                for ci in range(n_chunk):
                    nc.gpsimd.dma_gather(
                        out_ap=X[:, ci * ccols * e : (ci + 1) * ccols * e]
                        .rearrange("p (c e) -> p c e", e=e),
                        in_ap=stg[:],
                        idxs_ap=l2t[:, ci * icols : (ci + 1) * icols],
                        num_idxs=chunk,
                        num_idxs_reg=chunk,
                        elem_size=e,
                        single_packet=False,
                        queue_num=ci % N_QUEUES,
                    )

                if ABLATE < 4:
                    continue
                # --- masks into identical layout ---
                M = pool.tile([P, xcols * e], bf16, tag="M")
                m3 = M[:].rearrange("p (t r) -> p t r", t=sg_tiles)
                rows = slice(sg * sg_tiles * P, (sg + 1) * sg_tiles * P)
                nc.sync.dma_start(
                    out=m3[:, :, 0:e],
                    in_=mv[rows, :].rearrange("(t p) e -> p t e", p=P),
                )
                nc.sync.dma_start(
                    out=m3[:, :, e : 2 * e],
                    in_=mu[rows, :].rearrange("(t p) e -> p t e", p=P),
                )
                nc.sync.dma_start(
                    out=m3[:, :, 2 * e : slots * e],
                    in_=mn[rows, :].rearrange("(t p) r -> p t r", p=P),
                )

                if ABLATE < 5:
                    continue
                # --- compute per tile ---
                # negsum_b = <sum_k X_k, vm> (k-sum commutes past vm)
                xv = X[:]
                for tl in range(sg_tiles):
                    t = sg * sg_tiles + tl
                    base = tl * slots * e
                    nc.vector.tensor_tensor(
                        out=xv[:, base : base + slots * e],
                        in0=xv[:, base : base + slots * e],
                        in1=M[:][:, base : base + slots * e],
                        op=mult,
                    )
                    vm = xv[:, base : base + e]
                    # un[p, e] = sum_k X[p, 2+k, e]: strided reduce, inner
                    # axis = k (stride e), outer axis = e (stride 1)
                    un = pool.tile([P, e], f32, tag="un")
                    nc.vector.tensor_reduce(
                        out=un[:],
                        in_=xv[:, base + 2 * e : base + slots * e]
                        .rearrange("p (s e) -> p s e", e=e)
                        .transpose([0, 2, 1]),
                        axis=mybir.AxisListType.X,
                        op=add,
                    )
                    yp = pool.tile([P, e], f32, tag="yp")
                    nc.vector.tensor_tensor(
                        out=yp[:], in0=xv[:, base + e : base + 2 * e], in1=vm,
                        op=mult,
                    )
                    nc.scalar.activation(
                        out=yp[:], in_=yp[:], func=AF.Copy,
                        accum_out=post[:, t : t + 1],
                    )
                    yn = pool.tile([P, e], f32, tag="yn")
                    nc.vector.tensor_tensor(
                        out=yn[:], in0=un[:], in1=vm, op=mult
                    )
                    nc.scalar.activation(
                        out=yn[:], in_=yn[:], func=AF.Copy,
                        accum_out=negt[:, t : t + 1],
                    )

            if ABLATE < 5:
                nc.gpsimd.memset(post[:], 0.0)
                nc.gpsimd.memset(negt[:], 0.0)
            # --- softplus tail (f32): mean(softplus(-pos) + softplus(neg)) ---
            # softplus(z) = relu(z) + ln(1 + exp(-|z|))
            sabs = apool.tile([P, tiles], f32, tag="sabs")
            sexp = apool.tile([P, tiles], f32, tag="sexp")
            sln = apool.tile([P, tiles], f32, tag="sln")
            srel = apool.tile([P, tiles], f32, tag="srel")
            ssum = apool.tile([P, tiles], f32, tag="ssum")
            acc1 = apool.tile([P, 1], f32, tag="acc1")
            acc2 = apool.tile([P, 1], f32, tag="acc2")
            tot = apool.tile([P, 1], f32, tag="tot")

            for src, sgn, acc in ((post, -1.0, acc1), (negt, 1.0, acc2)):
                nc.scalar.activation(out=sabs[:], in_=src[:], func=AF.Abs)
                nc.scalar.activation(
                    out=sexp[:], in_=sabs[:], func=AF.Exp, scale=-1.0
                )
                nc.scalar.activation(out=sln[:], in_=sexp[:], func=AF.Ln, bias=1.0)
                nc.scalar.activation(
                    out=srel[:], in_=src[:], func=AF.Relu, scale=sgn
                )
                nc.vector.tensor_tensor(
                    out=ssum[:], in0=sln[:], in1=srel[:], op=add
                )
                nc.scalar.activation(
                    out=ssum[:], in_=ssum[:], func=AF.Copy, accum_out=acc[:]
                )
            nc.vector.tensor_tensor(out=tot[:], in0=acc1[:], in1=acc2[:], op=add)
            nc.sync.dma_start(out=out[:], in_=tot[:])

    nc.compile()
    return nc


def _get_program(bsh, vocab, k, e, sg_tiles, caps, ncores):
    key = (bsh, vocab, k, e, sg_tiles, caps, ncores)
    if key not in _prog_cache:
        _prog_cache[key] = _build_program(
            bsh, vocab, k, e, sg_tiles, caps, ncores
        )
    return _prog_cache[key]


def _wrap16(vals, ncols):
    """int16 list -> [128, ncols] tile data: value i at [i%16, i//16],
    replicated across the 8 16-partition groups."""
    assert vals.shape[0] == ncols * 16
    arr = np.ascontiguousarray(vals.reshape(ncols, 16).T)
    return np.tile(arr, (8, 1))


def _host_prep(
    ctx_words, target_words, neg_words, V_emb, U_emb, mask_v, mask_u, mask_neg,
    ncores, sg_tiles, caps,
):
    bf16 = _bf16()
    b, k = neg_words.shape
    vocab, e = V_emb.shape
    bsh = b // ncores
    slots = 2 + k
    tiles = bsh // P
    nsg = tiles // sg_tiles
    sg_pos = sg_tiles * P * slots
    trows = sum(caps)
    tcols = trows // P
    l1_cols = trows // 16
    l2_cols = sg_pos // 16
    nbuck = len(caps)

    W = np.concatenate(
        [np.asarray(V_emb, dtype=np.float32), np.asarray(U_emb, dtype=np.float32)],
        axis=0,
    ).astype(bf16)

    ctx = np.clip(np.asarray(ctx_words).reshape(b), 0, vocab - 1).astype(np.int64)
    tgt = np.clip(np.asarray(target_words).reshape(b), 0, vocab - 1).astype(np.int64)
    neg = np.clip(np.asarray(neg_words).reshape(b, k), 0, vocab - 1).astype(np.int64)

    # gather ids per position: ids[b_row, slot]
    ids_all = np.empty((b, slots), dtype=np.int32)
    ids_all[:, 0] = ctx
    ids_all[:, 1] = vocab + tgt
    ids_all[:, 2:] = vocab + neg

    mask_v = np.asarray(mask_v, dtype=np.float32).reshape(b, e).astype(bf16)
    mask_u = np.asarray(mask_u, dtype=np.float32).reshape(b, e).astype(bf16)
    mask_neg = (
        np.asarray(mask_neg, dtype=np.float32).reshape(b, k * e).astype(bf16)
    )

    cap_arr = np.asarray(caps, dtype=np.int64)
    cap_base = np.concatenate([[0], np.cumsum(cap_arr)])  # staging row base

    in_maps = []
    for c in range(ncores):
        lo = c * bsh
        l1_list = np.empty((nsg, trows), dtype=np.int16)
        l2_list = np.empty((nsg, sg_pos), dtype=np.int16)
        for sg in range(nsg):
            rlo = lo + sg * sg_tiles * P
            # position i = col*128 + p ; col = t_in_sg*slots + s
            idsl = ids_all[rlo : rlo + sg_tiles * P]          # [(t p), s]
            ids_pos = (
                idsl.reshape(sg_tiles, P, slots)
                .transpose(0, 2, 1)
                .reshape(-1)
            )  # index by (t, s, p) = position order ✓
            order = np.argsort(ids_pos, kind="stable")
            sids = ids_pos[order]
            bucket = sids >> 15  # // 32768
            counts = np.bincount(bucket, minlength=nbuck)
            if np.any(counts > cap_arr):
                raise RuntimeError(
                    f"bucket overflow: counts={counts} caps={caps}"
                )
            # L1 idx list: bucket-compact, padded to cap with idx -1
            # (trailing negative idxs are trimmed by the gather ucode:
            # no descriptor generation and no DMA traffic for padding)
            l1 = (
                np.full(trows, -1, dtype=np.int16)
                if PAD_NEG1
                else np.zeros(trows, dtype=np.int16)
            )
            # staging rank for each sorted element
            rank = np.empty(sg_pos, dtype=np.int64)
            off = 0
            for bi in range(nbuck):
                cnt = counts[bi]
                seg = sids[off : off + cnt] - bi * BUCKET
                l1[cap_base[bi] : cap_base[bi] + cnt] = seg.astype(np.int16)
                rank[off : off + cnt] = cap_base[bi] + np.arange(cnt)
                off += cnt
            # staging row of sorted element j (written by dma_gather at
            # (p=j%128, c=cap-col base...) -> stage-out maps (p,c) to row
            # p*tcols + c ; j within bucket block: p=rank%128, c=rank//128
            srow = (rank % P) * tcols + (rank // P)
            l2 = np.empty(sg_pos, dtype=np.int16)
            l2[order] = srow.astype(np.int16)
            l1_list[sg] = l1
            l2_list[sg] = l2
        l1m = np.concatenate(
            [_wrap16(l1_list[sg], l1_cols) for sg in range(nsg)], axis=1
        )
        l2m = np.concatenate(
            [_wrap16(l2_list[sg], l2_cols) for sg in range(nsg)], axis=1
        )
        in_maps.append(
            {
                "w": W,
                "l1i": l1m,
                "l2i": l2m,
                "mv": mask_v[lo : lo + bsh],
                "mu": mask_u[lo : lo + bsh],
                "mn": mask_neg[lo : lo + bsh],
            }
        )
    return in_maps


def kernel(
    ctx_words, target_words, neg_words, V_emb, U_emb, mask_v, mask_u, mask_neg
):
    from concourse.bass_utils import run_bass_kernel_spmd

    b, k = neg_words.shape
    vocab, e = V_emb.shape
    bsh = b // NCORES

    nc = _get_program(bsh, vocab, k, e, SG_TILES, CAPS, NCORES)
    in_maps = _host_prep(
        ctx_words, target_words, neg_words, V_emb, U_emb,
        mask_v, mask_u, mask_neg, NCORES, SG_TILES, CAPS,
    )
    res = run_bass_kernel_spmd(nc, in_maps, core_ids=list(range(NCORES)))
    total = np.float64(0.0)
    for c in range(NCORES):
        total += np.float64(
            res.results[c]["out"].astype(np.float64).sum()
        )
    return np.float32(total / b)



# revision 14
# speedup vs baseline: 2.1271x; 2.1271x over previous
"""CBOW negative-sampling loss on 8 Trainium2 NeuronCores.

Reference computation:
    v      = V_emb[ctx] * mask_v                  # [B,1,E]
    u      = U_emb[tgt] * mask_u                  # [B,1,E]
    u_neg  = -(U_emb[neg] * mask_neg)             # [B,K,E]
    pos    = <u, v>
    neg    = sum_k <u_neg_k, v>
    loss   = -mean(log_sigmoid(pos) + log_sigmoid(neg))
           = mean(softplus(-pos) + softplus(negsum)),  negsum = -neg

Strategy: data-parallel over B across 8 cores, tables replicated.  Both
embedding tables are concatenated host-side into one [2*VOCAB, E] table
(cast to bf16) so each batch row needs 12 row-gathers (ctx, tgt, 10 neg)
from a single table.  The only high-throughput gather on TRN2 is the
GPSIMD dma_gather (mlp library), whose indices are int16 (< 32768) —
too small for 200k rows.  So gathers are two-level, per "super group"
(sg) of 8 row-tiles = 12288 gather positions:

  L1: the sg's gather ids are sorted and split into 7 static-capacity
      range buckets; per bucket one compact dma_gather call (id - base
      fits int16) pulls the rows into an SBUF tile in sorted order.
  stage: one contiguous 128-descriptor DMA copies that tile to a DRAM
      scratch laid out so sorted rank r lives at scratch row
      r = partition*TCOLS + column (per-partition contiguous).
  L2: one positional dma_gather (indices = staging rank < 14080, int16)
      scatters all 12288 rows into batch-position layout [p, col*E:..].

Masks stream into an identically-laid-out tile; the DVE multiplies
X*M, multiplies by the broadcast masked-ctx row (vm), and the scalar
engine accumulates per-tile pos/neg dot sums (ACT Copy + accum_out).
Final softplus tail in f32; per-core [128,1] partial sums are reduced
on the host.  bf16 storage throughout the bulk path: the final value
is a mean over 65536 rows, so per-row rounding noise averages out
(measured ~1e-4 relative).
"""

import numpy as np

B, K, VOCAB, E = 65536, 10, 100000, 128
NCORES = 8
P = 128
SLOTS = 2 + K

# two-level gather geometry
SG_TILES = 8                       # row-tiles per super group
SG_POS = SG_TILES * P * SLOTS      # gather positions per sg = 12288
BUCKET = 32768
# static per-bucket capacities (rows, multiples of 128) sized for uniform
# ids: slot0 uniform [0,100000), slots1-11 uniform [100000,200000)
CAPS = (512, 512, 512, 3840, 4096, 4096, 512)
TROWS = sum(CAPS)                  # 14080 staging rows per sg
TCOLS = TROWS // P                 # 110

_prog_cache = {}

# debug ablation: 1=L1 only, 2=+stage, 3=+L2, 4=+masks, 5=+compute, 6=full
ABLATE = 6
# number of SWDGE queues (GPSIMD core pairs) to spread gathers over, 1-4
N_QUEUES = 4
# pad L1 idx lists with trailing -1 (trimmed by ucode) instead of 0
PAD_NEG1 = False
# split the L2 gather into this many chunks (round-robin over queues)
L2_SPLIT = 4


def _bf16():
    import ml_dtypes

    return np.dtype(ml_dtypes.bfloat16)


def _build_program(bsh, vocab, k, e, sg_tiles, caps, ncores):
    import concourse.bacc as bacc
    import concourse.tile as tile
    from concourse import library_config, mybir

    f32 = mybir.dt.float32
    bf16 = mybir.dt.bfloat16
    i16 = mybir.dt.int16
    slots = 2 + k
    tiles = bsh // P
    assert tiles % sg_tiles == 0
    nsg = tiles // sg_tiles
    sg_pos = sg_tiles * P * slots
    trows = sum(caps)
    tcols = trows // P
    xcols = sg_tiles * slots           # X columns per sg
    l1_cols = trows // 16              # int16 idx columns per sg (16-wrap)
    l2_cols = sg_pos // 16

    nc = bacc.Bacc(
        "TRN2", target_bir_lowering=False, debug=False, num_devices=ncores,
        num_swdge_queues=N_QUEUES,
    )
    w = nc.dram_tensor("w", [2 * vocab, e], bf16, kind="ExternalInput")
    l1i = nc.dram_tensor("l1i", [P, nsg * l1_cols], i16, kind="ExternalInput")
    l2i = nc.dram_tensor("l2i", [P, nsg * l2_cols], i16, kind="ExternalInput")
    mv = nc.dram_tensor("mv", [bsh, e], bf16, kind="ExternalInput")
    mu = nc.dram_tensor("mu", [bsh, e], bf16, kind="ExternalInput")
    mn = nc.dram_tensor("mn", [bsh, k * e], bf16, kind="ExternalInput")
    out = nc.dram_tensor("out", [P, 1], f32, kind="ExternalOutput")

    mult = mybir.AluOpType.mult
    add = mybir.AluOpType.add
    AF = mybir.ActivationFunctionType

    with tile.TileContext(nc) as tc:
        with (
            tc.tile_pool(name="sb", bufs=2) as pool,
            tc.tile_pool(name="stg", bufs=2, space="DRAM") as dpool,
            tc.tile_pool(name="acc", bufs=1) as apool,
        ):
            nc.gpsimd.load_library(library_config.mlp)

            post = apool.tile([P, tiles], f32, tag="post")
            negt = apool.tile([P, tiles], f32, tag="negt")

            for sg in range(nsg):
                # --- index tiles for this sg ---
                l1t = pool.tile([P, l1_cols], i16, tag="l1t")
                nc.sync.dma_start(
                    out=l1t[:], in_=l1i[:, sg * l1_cols : (sg + 1) * l1_cols]
                )
                l2t = pool.tile([P, l2_cols], i16, tag="l2t")
                nc.sync.dma_start(
                    out=l2t[:], in_=l2i[:, sg * l2_cols : (sg + 1) * l2_cols]
                )

                # --- L1: bucketed compact gathers into sorted-order tile ---
                # Buckets are spread across the 4 SWDGE queues (each queue =
                # its own GPSIMD core pair generating descriptors), so the
                # per-bucket generation runs concurrently.  Trailing -1
                # padding in the idx lists is trimmed by the ucode before
                # descriptor generation, so padding costs nothing.
                T = pool.tile([P, tcols * e], bf16, tag="T")
                colbase = 0
                # big buckets 3,4,5 get their own queue; small ones share q3
                bq = {0: 3, 1: 3, 2: 3, 3: 0, 4: 1, 5: 2, 6: 3}
                for bi, cap in enumerate(caps):
                    nc.gpsimd.dma_gather(
                        out_ap=T[:, colbase * e : (colbase + cap // P) * e]
                        .rearrange("p (c e) -> p c e", e=e),
                        in_ap=w[bi * BUCKET :, :],
                        idxs_ap=l1t[:, colbase * 8 : (colbase + cap // P) * 8],
                        num_idxs=cap,
                        num_idxs_reg=cap,
                        elem_size=e,
                        single_packet=False,
                        queue_num=bq[bi] % N_QUEUES,
                    )
                    colbase += cap // P

                if ABLATE < 2:
                    continue
                # --- stage to DRAM: sorted rank r = p*tcols + c ---
                stg = dpool.tile([trows, e], bf16, tag="stg")
                nc.sync.dma_start(
                    out=stg[:]
                    .rearrange("(p c) e -> p c e", p=P)
                    .rearrange("p c e -> p (c e)"),
                    in_=T[:],
                )

                if ABLATE < 3:
                    continue
                # --- L2: positional gather into batch layout ---
                # split into 4 chunks, one per SWDGE queue, for concurrent
                # descriptor generation
                X = pool.tile([P, xcols * e], bf16, tag="X")
                n_chunk = L2_SPLIT
                chunk = sg_pos // n_chunk          # 3072 positions
                ccols = chunk // P                 # 24 output cols of e
                icols = chunk // 16                # 192 idx cols
                for ci in range(n_chunk):
                    nc.gpsimd.dma_gather(
                        out_ap=X[:, ci * ccols * e : (ci + 1) * ccols * e]
                        .rearrange("p (c e) -> p c e", e=e),
                        in_ap=stg[:],
                        idxs_ap=l2t[:, ci * icols : (ci + 1) * icols],
                        num_idxs=chunk,
                        num_idxs_reg=chunk,
                        elem_size=e,
                        single_packet=False,
                        queue_num=ci % N_QUEUES,
                    )

                if ABLATE < 4:
                    continue
                # --- masks into identical layout ---
                M = pool.tile([P, xcols * e], bf16, tag="M")
                m3 = M[:].rearrange("p (t r) -> p t r", t=sg_tiles)
                rows = slice(sg * sg_tiles * P, (sg + 1) * sg_tiles * P)
                nc.sync.dma_start(
                    out=m3[:, :, 0:e],
                    in_=mv[rows, :].rearrange("(t p) e -> p t e", p=P),
                )
                nc.sync.dma_start(
                    out=m3[:, :, e : 2 * e],
                    in_=mu[rows, :].rearrange("(t p) e -> p t e", p=P),
                )
                nc.sync.dma_start(
                    out=m3[:, :, 2 * e : slots * e],
                    in_=mn[rows, :].rearrange("(t p) r -> p t r", p=P),
                )

                if ABLATE < 5:
                    continue
                # --- compute per tile ---
                # negsum_b = <sum_k X_k, vm> (k-sum commutes past vm)
                xv = X[:]
                for tl in range(sg_tiles):
                    t = sg * sg_tiles + tl
                    base = tl * slots * e
                    nc.vector.tensor_tensor(
                        out=xv[:, base : base + slots * e],
                        in0=xv[:, base : base + slots * e],
                        in1=M[:][:, base : base + slots * e],
                        op=mult,
                    )
                    vm = xv[:, base : base + e]
                    # un[p, e] = sum_k X[p, 2+k, e]: strided reduce, inner
                    # axis = k (stride e), outer axis = e (stride 1)
                    un = pool.tile([P, e], f32, tag="un")
                    nc.vector.tensor_reduce(
                        out=un[:],
                        in_=xv[:, base + 2 * e : base + slots * e]
                        .rearrange("p (s e) -> p s e", e=e)
                        .transpose([0, 2, 1]),
                        axis=mybir.AxisListType.X,
                        op=add,
                    )
                    yp = pool.tile([P, e], f32, tag="yp")
                    nc.vector.tensor_tensor(
                        out=yp[:], in0=xv[:, base + e : base + 2 * e], in1=vm,
                        op=mult,
                    )
                    nc.scalar.activation(
                        out=yp[:], in_=yp[:], func=AF.Copy,
                        accum_out=post[:, t : t + 1],
                    )
                    yn = pool.tile([P, e], f32, tag="yn")
                    nc.vector.tensor_tensor(
                        out=yn[:], in0=un[:], in1=vm, op=mult
                    )
                    nc.scalar.activation(
                        out=yn[:], in_=yn[:], func=AF.Copy,
                        accum_out=negt[:, t : t + 1],
                    )

            if ABLATE < 5:
                nc.gpsimd.memset(post[:], 0.0)
                nc.gpsimd.memset(negt[:], 0.0)
            # --- softplus tail (f32): mean(softplus(-pos) + softplus(neg)) ---
            # softplus(z) = relu(z) + ln(1 + exp(-|z|))
            sabs = apool.tile([P, tiles], f32, tag="sabs")
            sexp = apool.tile([P, tiles], f32, tag="sexp")
            sln = apool.tile([P, tiles], f32, tag="sln")
            srel = apool.tile([P, tiles], f32, tag="srel")
            ssum = apool.tile([P, tiles], f32, tag="ssum")
            acc1 = apool.tile([P, 1], f32, tag="acc1")
            acc2 = apool.tile([P, 1], f32, tag="acc2")
            tot = apool.tile([P, 1], f32, tag="tot")

            for src, sgn, acc in ((post, -1.0, acc1), (negt, 1.0, acc2)):
                nc.scalar.activation(out=sabs[:], in_=src[:], func=AF.Abs)
                nc.scalar.activation(
                    out=sexp[:], in_=sabs[:], func=AF.Exp, scale=-1.0
                )
                nc.scalar.activation(out=sln[:], in_=sexp[:], func=AF.Ln, bias=1.0)
                nc.scalar.activation(
                    out=srel[:], in_=src[:], func=AF.Relu, scale=sgn
                )
                nc.vector.tensor_tensor(
                    out=ssum[:], in0=sln[:], in1=srel[:], op=add
                )
                nc.scalar.activation(
                    out=ssum[:], in_=ssum[:], func=AF.Copy, accum_out=acc[:]
                )
            nc.vector.tensor_tensor(out=tot[:], in0=acc1[:], in1=acc2[:], op=add)
            nc.sync.dma_start(out=out[:], in_=tot[:])

    nc.compile()
    return nc


def _get_program(bsh, vocab, k, e, sg_tiles, caps, ncores):
    key = (bsh, vocab, k, e, sg_tiles, caps, ncores)
    if key not in _prog_cache:
        _prog_cache[key] = _build_program(
            bsh, vocab, k, e, sg_tiles, caps, ncores
        )
    return _prog_cache[key]


def _wrap16(vals, ncols):
    """int16 list -> [128, ncols] tile data: value i at [i%16, i//16],
    replicated across the 8 16-partition groups."""
    assert vals.shape[0] == ncols * 16
    arr = np.ascontiguousarray(vals.reshape(ncols, 16).T)
    return np.tile(arr, (8, 1))


def _host_prep(
    ctx_words, target_words, neg_words, V_emb, U_emb, mask_v, mask_u, mask_neg,
    ncores, sg_tiles, caps,
):
    bf16 = _bf16()
    b, k = neg_words.shape
    vocab, e = V_emb.shape
    bsh = b // ncores
    slots = 2 + k
    tiles = bsh // P
    nsg = tiles // sg_tiles
    sg_pos = sg_tiles * P * slots
    trows = sum(caps)
    tcols = trows // P
    l1_cols = trows // 16
    l2_cols = sg_pos // 16
    nbuck = len(caps)

    W = np.concatenate(
        [np.asarray(V_emb, dtype=np.float32), np.asarray(U_emb, dtype=np.float32)],
        axis=0,
    ).astype(bf16)

    ctx = np.clip(np.asarray(ctx_words).reshape(b), 0, vocab - 1).astype(np.int64)
    tgt = np.clip(np.asarray(target_words).reshape(b), 0, vocab - 1).astype(np.int64)
    neg = np.clip(np.asarray(neg_words).reshape(b, k), 0, vocab - 1).astype(np.int64)

    # gather ids per position: ids[b_row, slot]
    ids_all = np.empty((b, slots), dtype=np.int32)
    ids_all[:, 0] = ctx
    ids_all[:, 1] = vocab + tgt
    ids_all[:, 2:] = vocab + neg

    mask_v = np.asarray(mask_v, dtype=np.float32).reshape(b, e).astype(bf16)
    mask_u = np.asarray(mask_u, dtype=np.float32).reshape(b, e).astype(bf16)
    mask_neg = (
        np.asarray(mask_neg, dtype=np.float32).reshape(b, k * e).astype(bf16)
    )

    cap_arr = np.asarray(caps, dtype=np.int64)
    cap_base = np.concatenate([[0], np.cumsum(cap_arr)])  # staging row base

    in_maps = []
    for c in range(ncores):
        lo = c * bsh
        l1_list = np.empty((nsg, trows), dtype=np.int16)
        l2_list = np.empty((nsg, sg_pos), dtype=np.int16)
        for sg in range(nsg):
            rlo = lo + sg * sg_tiles * P
            # position i = col*128 + p ; col = t_in_sg*slots + s
            idsl = ids_all[rlo : rlo + sg_tiles * P]          # [(t p), s]
            ids_pos = (
                idsl.reshape(sg_tiles, P, slots)
                .transpose(0, 2, 1)
                .reshape(-1)
            )  # index by (t, s, p) = position order ✓
            order = np.argsort(ids_pos, kind="stable")
            sids = ids_pos[order]
            bucket = sids >> 15  # // 32768
            counts = np.bincount(bucket, minlength=nbuck)
            if np.any(counts > cap_arr):
                raise RuntimeError(
                    f"bucket overflow: counts={counts} caps={caps}"
                )
            # L1 idx list: bucket-compact, padded to cap with idx -1
            # (trailing negative idxs are trimmed by the gather ucode:
            # no descriptor generation and no DMA traffic for padding)
            l1 = (
                np.full(trows, -1, dtype=np.int16)
                if PAD_NEG1
                else np.zeros(trows, dtype=np.int16)
            )
            # staging rank for each sorted element
            rank = np.empty(sg_pos, dtype=np.int64)
            off = 0
            for bi in range(nbuck):
                cnt = counts[bi]
                seg = sids[off : off + cnt] - bi * BUCKET
                l1[cap_base[bi] : cap_base[bi] + cnt] = seg.astype(np.int16)
                rank[off : off + cnt] = cap_base[bi] + np.arange(cnt)
                off += cnt
            # staging row of sorted element j (written by dma_gather at
            # (p=j%128, c=cap-col base...) -> stage-out maps (p,c) to row
            # p*tcols + c ; j within bucket block: p=rank%128, c=rank//128
            srow = (rank % P) * tcols + (rank // P)
            l2 = np.empty(sg_pos, dtype=np.int16)
            l2[order] = srow.astype(np.int16)
            l1_list[sg] = l1
            l2_list[sg] = l2
        l1m = np.concatenate(
            [_wrap16(l1_list[sg], l1_cols) for sg in range(nsg)], axis=1
        )
        l2m = np.concatenate(
            [_wrap16(l2_list[sg], l2_cols) for sg in range(nsg)], axis=1
        )
        in_maps.append(
            {
                "w": W,
                "l1i": l1m,
                "l2i": l2m,
                "mv": mask_v[lo : lo + bsh],
                "mu": mask_u[lo : lo + bsh],
                "mn": mask_neg[lo : lo + bsh],
            }
        )
    return in_maps


def kernel(
    ctx_words, target_words, neg_words, V_emb, U_emb, mask_v, mask_u, mask_neg
):
    from concourse.bass_utils import run_bass_kernel_spmd

    b, k = neg_words.shape
    vocab, e = V_emb.shape
    bsh = b // NCORES

    nc = _get_program(bsh, vocab, k, e, SG_TILES, CAPS, NCORES)
    in_maps = _host_prep(
        ctx_words, target_words, neg_words, V_emb, U_emb,
        mask_v, mask_u, mask_neg, NCORES, SG_TILES, CAPS,
    )
    res = run_bass_kernel_spmd(nc, in_maps, core_ids=list(range(NCORES)))
    total = np.float64(0.0)
    for c in range(NCORES):
        total += np.float64(
            res.results[c]["out"].astype(np.float64).sum()
        )
    return np.float32(total / b)



# revision 52
# speedup vs baseline: 3.7518x; 1.7638x over previous
"""CBOW negative-sampling loss on 8 Trainium2 NeuronCores.

Reference computation:
    v      = V_emb[ctx] * mask_v                  # [B,1,E]
    u      = U_emb[tgt] * mask_u                  # [B,1,E]
    u_neg  = -(U_emb[neg] * mask_neg)             # [B,K,E]
    pos    = <u, v>
    neg    = sum_k <u_neg_k, v>
    loss   = -mean(log_sigmoid(pos) + log_sigmoid(neg))
           = mean(softplus(-pos) + softplus(negsum)),  negsum = -neg

Strategy: data-parallel over B across 8 cores, tables replicated.  Both
embedding tables are concatenated host-side into one [2*VOCAB, E] table
(cast to bf16) so each batch row needs 12 row-gathers (ctx, tgt, 10 neg)
from a single table.  The only high-throughput gather on TRN2 is the
GPSIMD dma_gather (mlp library), whose indices are int16 (< 32768) --
too small for 200k rows.  So gathers are two-level, per "super group"
(sg) of 8 row-tiles = 12288 gather positions:

  L1: the sg's gather ids are sorted and split into range sub-buckets
      (id - 32768*bucket fits int16); per sub-bucket one compact
      dma_gather pulls the rows into an SBUF tile in sorted order.
  stage: one contiguous 128-descriptor DMA copies that tile to a DRAM
      scratch laid out so gather position j lives at scratch row
      (j % 128) * TCOLS + j // 128 (per-partition contiguous).
  L2: positional dma_gathers (indices = staging row < TROWS, int16)
      scatter all 12288 rows into batch-position layout [p, col*E:..].

GPSIMD descriptor generation is the bottleneck (~8ns/row on one core
pair), so the calls are spread over all 4 SWDGE queues (= 4 GPSIMD core
pairs generating concurrently), load-balanced by expected row count.
Idx lists are padded with trailing -1, which the gather ucode trims
before generation; num_idxs_reg carries the exact per-call count
(loaded from an input tensor at runtime) so the decode-side ring
bookkeeping agrees with the trimmed count.

mask_v is folded into mask_u/mask_neg on the host (m'_s = m_s * m_v),
so slot 0 (the ctx row) stays raw and each tile's scores reduce to
    pos[p]  = sum_e  X[s=1]*m'_u * X[s=0]
    negs[p] = sum_ke X[s=2+k]*m'_k * X[s=0]
computed per tile by an in-place mask multiply + a pairwise add tree
over contiguous slot slices (2x cheaper than the strided tensor_reduce)
+ two small multiplies on the DVE, with the e-dots accumulated by the
scalar engine (ACT Copy + accum_out).  (tensor_tensor_reduce would
fuse this but crashes the device on this HW stack.)  Masks travel as
fp8 (values {0,4} are exact) and are upconverted to bf16 per tile by
the otherwise-idle scalar engine so the DVE keeps its 2x 16-bit rate.
Final softplus tail in f32; per-core [128,1] partial sums are reduced
on the host.  bf16 storage on the gather path: the final value is a
mean over 65536 rows, so rounding noise averages out (~1e-5 measured).
"""

import numpy as np

B, K, VOCAB, E = 65536, 10, 100000, 128
NCORES = 8
P = 128
SLOTS = 2 + K

# two-level gather geometry
SG_TILES = 8                       # row-tiles per super group
SG_POS = SG_TILES * P * SLOTS      # gather positions per sg = 12288
BUCKET = 32768
# (bucket, capacity) sub-buckets; capacities are multiples of 128 and
# sized from the actual per-sg count distribution (means ~336/336/334/
# 3523/3698/3679/382, sigma ~50) with >5-sigma margin.  Big buckets are
# split in two so their descriptor generation spreads across queues.
SUBBUCKETS = (
    (0, 384), (1, 384), (2, 384),
    (3, 1920), (3, 1792),
    (4, 1920), (4, 1920),
    (5, 1920), (5, 1920),
    (6, 512),
)
# Each of the 8 DMASW semaphore lanes may only be incremented from one
# SWDGE queue, and at most 8 pool-DMA instructions can be outstanding
# (call N waits for call N-8's DMA completion when its lane recycles).
# Fewer, bigger calls (12/sg) keep the 8-deep window ~0.7 sg wide, and
# the post-scheduling pass below maps lanes->queues to balance load.
CALL_ORDER = (5, 7, 6, 8, 4, 3, 0, 1, 9, 2)  # L1 order, big interleaved
# L2 positional gather chunks (multiples of 128, sum 12288)
L2_CHUNKS = (2048, 2048, 2048, 2048, 2048, 2048)
# expected gathered rows per call, keyed by static num_idxs, for the
# lane->queue balancing (L1 counts are means of the actual distribution)
EXPECT_ROWS = {384: 335, 512: 382, 1792: 1603, 1920: 1859, 2048: 2048}
CAPS = SUBBUCKETS                  # kept name for test.py compatibility
TROWS = sum(c for _, c in SUBBUCKETS)   # 13056 staging rows per sg
TCOLS = TROWS // P                 # 102
NCALLS = len(SUBBUCKETS)

_prog_cache = {}

# debug ablation: 1=L1 only, 2=+stage, 3=+L2, 4=+masks, 5=+compute, 6=full
ABLATE = 5


def _bf16():
    import ml_dtypes

    return np.dtype(ml_dtypes.bfloat16)


def _build_program(bsh, vocab, k, e, sg_tiles, caps, ncores):
    import concourse.bacc as bacc
    import concourse.tile as tile
    from concourse import library_config, mybir

    f32 = mybir.dt.float32
    bf16 = mybir.dt.bfloat16
    i16 = mybir.dt.int16
    i32 = mybir.dt.int32
    slots = 2 + k
    mslots = slots - 1                 # mask slots (ctx mask folded away)
    tiles = bsh // P
    assert tiles % sg_tiles == 0
    nsg = tiles // sg_tiles
    sg_pos = sg_tiles * P * slots
    trows = TROWS
    tcols = trows // P
    xcols = sg_tiles * slots           # X columns per sg
    mcols = sg_tiles * mslots          # mask columns per sg
    l1_cols = trows // 16              # int16 idx columns per sg (16-wrap)
    l2_cols = sg_pos // 16

    nc = bacc.Bacc(
        "TRN2", target_bir_lowering=False, debug=False, num_devices=ncores,
        num_swdge_queues=4,
    )
    w = nc.dram_tensor("w", [2 * vocab, e], bf16, kind="ExternalInput")
    l1i = nc.dram_tensor("l1i", [P, nsg * l1_cols], i16, kind="ExternalInput")
    l2i = nc.dram_tensor("l2i", [P, nsg * l2_cols], i16, kind="ExternalInput")
    cnt = nc.dram_tensor("cnt", [P, nsg * NCALLS], i32, kind="ExternalInput")
    f8 = mybir.dt.float8e4
    mu = nc.dram_tensor("mu", [bsh, e], f8, kind="ExternalInput")
    mn = nc.dram_tensor("mn", [bsh, k * e], f8, kind="ExternalInput")
    out = nc.dram_tensor("out", [P, 1], f32, kind="ExternalOutput")

    mult = mybir.AluOpType.mult
    add = mybir.AluOpType.add
    AF = mybir.ActivationFunctionType

    with tile.TileContext(nc) as tc:
        with (
            tc.tile_pool(name="sb", bufs=2) as pool,
            tc.tile_pool(name="stg", bufs=2, space="DRAM") as dpool,
            tc.tile_pool(name="acc", bufs=1) as apool,
        ):
            nc.gpsimd.load_library(library_config.mlp)

            post = apool.tile([P, tiles], f32, tag="post")
            negt = apool.tile([P, tiles], f32, tag="negt")
            cntt = apool.tile([P, nsg * NCALLS], i32, tag="cntt")
            nc.sync.dma_start(out=cntt[:], in_=cnt[:, :])
            # prefetch ALL per-sg index tiles once: keeps sync-engine DMAs
            # (and their waits) out of the Pool engine's instruction stream
            l1all = apool.tile([P, nsg * l1_cols], i16, tag="l1all")
            nc.sync.dma_start(out=l1all[:], in_=l1i[:, :])
            l2all = apool.tile([P, nsg * l2_cols], i16, tag="l2all")
            nc.sync.dma_start(out=l2all[:], in_=l2i[:, :])
            # one count register per sub-bucket slot, reused across sgs
            # (engine front-end decode is in program order, so the reload
            # for sg n+1 cannot clobber the value before the sg n gather
            # decoded it)
            cnt_regs = [
                nc.gpsimd.alloc_register(f"cnt_reg_{si}")
                for si in range(NCALLS)
            ]

            sub_cols = [0]
            for _, cap_ in SUBBUCKETS:
                sub_cols.append(sub_cols[-1] + cap_ // P)

            def emit_front(sg):
                """L1 gathers + stage for supergroup sg."""
                T = pool.tile([P, tcols * e], bf16, tag="T")
                # one batched load fills all NCALLS count registers for
                # this sg (values are consecutive in cntt)
                nc.gpsimd.reg_load(
                    cnt_regs,
                    cntt[0:1, sg * NCALLS : (sg + 1) * NCALLS],
                )
                for pos, si in enumerate(CALL_ORDER):
                    bi, cap = SUBBUCKETS[si]
                    cb = sub_cols[si]
                    nc.gpsimd.dma_gather(
                        out_ap=T[:, cb * e : (cb + cap // P) * e]
                        .rearrange("p (c e) -> p c e", e=e),
                        in_ap=w[bi * BUCKET :, :],
                        idxs_ap=l1all[
                            :,
                            sg * l1_cols + cb * 8 :
                            sg * l1_cols + (cb + cap // P) * 8,
                        ],
                        num_idxs=cap,
                        num_idxs_reg=cnt_regs[si],
                        elem_size=e,
                        single_packet=False,
                        queue_num=0,
                    )
                if ABLATE < 2:
                    return None
                # stage to DRAM: position j -> row (j%128)*tcols + j//128
                stg = dpool.tile([trows, e], bf16, tag="stg")
                nc.sync.dma_start(
                    out=stg[:]
                    .rearrange("(p c) e -> p c e", p=P)
                    .rearrange("p c e -> p (c e)"),
                    in_=T[:],
                )
                # masks have no deps: load them here so they are long ready
                # by the time this sg's compute runs (keeps them off the
                # critical tail)
                M = pool.tile([P, mcols * e], f8, tag="M")
                m3 = M[:].rearrange("p (t r) -> p t r", t=sg_tiles)
                rows = slice(sg * sg_tiles * P, (sg + 1) * sg_tiles * P)
                nc.sync.dma_start(
                    out=m3[:, :, 0:e],
                    in_=mu[rows, :].rearrange("(t p) e -> p t e", p=P),
                )
                nc.sync.dma_start(
                    out=m3[:, :, e : mslots * e],
                    in_=mn[rows, :].rearrange("(t p) r -> p t r", p=P),
                )
                return (sg, stg, M)

            def emit_back(state):
                """L2 gathers + masks + per-tile compute for a staged sg."""
                if state is None or ABLATE < 3:
                    return
                sg, stg, M = state
                X = pool.tile([P, xcols * e], bf16, tag="X")
                pos0 = 0
                for ci, chunk in enumerate(L2_CHUNKS):
                    ccols = chunk // P
                    icols = chunk // 16
                    ob = pos0 // P
                    ib = pos0 // 16
                    nc.gpsimd.dma_gather(
                        out_ap=X[:, ob * e : (ob + ccols) * e]
                        .rearrange("p (c e) -> p c e", e=e),
                        in_ap=stg[:],
                        idxs_ap=l2all[
                            :, sg * l2_cols + ib : sg * l2_cols + ib + icols
                        ],
                        num_idxs=chunk,
                        num_idxs_reg=chunk,
                        elem_size=e,
                        single_packet=False,
                        queue_num=0,
                    )
                    pos0 += chunk

                if ABLATE < 5:
                    return
                # per-tile compute
                xv = X[:]
                for tl in range(sg_tiles):
                    t = sg * sg_tiles + tl
                    xb = tl * slots * e
                    mb = tl * mslots * e
                    # masks travel as fp8 (values {0,4} exact; halves their
                    # DMA traffic); the idle scalar engine upconverts so
                    # the DVE keeps its 2x 16-bit rate
                    Mb = pool.tile([P, mslots * e], bf16, tag="Mb")
                    nc.scalar.activation(
                        out=Mb[:],
                        in_=M[:][:, mb : mb + mslots * e],
                        func=AF.Copy,
                    )
                    # in-place mask multiply on slots 1..11 (slot 0 raw)
                    nc.vector.tensor_tensor(
                        out=xv[:, xb + e : xb + slots * e],
                        in0=xv[:, xb + e : xb + slots * e],
                        in1=Mb[:],
                        op=mult,
                    )
                    v0 = xv[:, xb : xb + e]
                    # un[p, e] = sum_k (u_k * m'_k): pairwise add tree on
                    # contiguous slices (the strided tensor_reduce costs
                    # ~2x more DVE time)
                    pw = pool.tile([P, 5 * e], bf16, tag="pw")
                    nc.vector.tensor_tensor(
                        out=pw[:],
                        in0=xv[:, xb + 2 * e : xb + 7 * e],
                        in1=xv[:, xb + 7 * e : xb + slots * e],
                        op=add,
                    )
                    nc.vector.tensor_tensor(
                        out=pw[:, 0 : 2 * e],
                        in0=pw[:, 0 : 2 * e],
                        in1=pw[:, 2 * e : 4 * e],
                        op=add,
                    )
                    nc.vector.tensor_tensor(
                        out=pw[:, 0:e],
                        in0=pw[:, 0:e],
                        in1=pw[:, e : 2 * e],
                        op=add,
                    )
                    un = pool.tile([P, e], f32, tag="un")
                    nc.vector.tensor_tensor(
                        out=un[:],
                        in0=pw[:, 0:e],
                        in1=pw[:, 4 * e : 5 * e],
                        op=add,
                    )
                    yp = pool.tile([P, e], f32, tag="yp")
                    nc.vector.tensor_tensor(
                        out=yp[:], in0=xv[:, xb + e : xb + 2 * e], in1=v0,
                        op=mult,
                    )
                    nc.scalar.activation(
                        out=yp[:], in_=yp[:], func=AF.Copy,
                        accum_out=post[:, t : t + 1],
                    )
                    yn = pool.tile([P, e], f32, tag="yn")
                    nc.vector.tensor_tensor(
                        out=yn[:], in0=un[:], in1=v0, op=mult
                    )
                    nc.scalar.activation(
                        out=yn[:], in_=yn[:], func=AF.Copy,
                        accum_out=negt[:, t : t + 1],
                    )

            # Software-pipelined emission: issue L1(sg+1) BEFORE L2(sg) so
            # the Pool engine's in-order instruction stream never stalls
            # head-of-line on stage(sg) completing -- by the time L2(sg)
            # reaches the front, its staging DMA has finished behind L1(sg+1).
            prev = None
            for sg in range(nsg):
                cur = emit_front(sg)
                if prev is not None:
                    emit_back(prev)
                prev = cur
            emit_back(prev)

            if ABLATE < 5:
                nc.gpsimd.memset(post[:], 0.0)
                nc.gpsimd.memset(negt[:], 0.0)
            # --- softplus tail (f32): mean(softplus(-pos) + softplus(neg)) ---
            # softplus(z) = relu(z) + ln(1 + exp(-|z|))
            sabs = apool.tile([P, tiles], f32, tag="sabs")
            sexp = apool.tile([P, tiles], f32, tag="sexp")
            sln = apool.tile([P, tiles], f32, tag="sln")
            srel = apool.tile([P, tiles], f32, tag="srel")
            ssum = apool.tile([P, tiles], f32, tag="ssum")
            acc1 = apool.tile([P, 1], f32, tag="acc1")
            acc2 = apool.tile([P, 1], f32, tag="acc2")
            tot = apool.tile([P, 1], f32, tag="tot")

            for src, sgn, acc in ((post, -1.0, acc1), (negt, 1.0, acc2)):
                nc.scalar.activation(out=sabs[:], in_=src[:], func=AF.Abs)
                nc.scalar.activation(
                    out=sexp[:], in_=sabs[:], func=AF.Exp, scale=-1.0
                )
                nc.scalar.activation(out=sln[:], in_=sexp[:], func=AF.Ln, bias=1.0)
                nc.scalar.activation(
                    out=srel[:], in_=src[:], func=AF.Relu, scale=sgn
                )
                nc.vector.tensor_tensor(
                    out=ssum[:], in0=sln[:], in1=srel[:], op=add
                )
                nc.scalar.activation(
                    out=ssum[:], in_=ssum[:], func=AF.Copy, accum_out=acc[:]
                )
            nc.vector.tensor_tensor(out=tot[:], in0=acc1[:], in1=acc2[:], op=add)
            nc.sync.dma_start(out=out[:], in_=tot[:])

    # --- align gather queue_num with the scheduler's DMASW lane ---
    # The tile scheduler assigns Pool-engine DMA instructions to the 8
    # DMASW semaphore lanes round-robin in SCHEDULED order, and a lane's
    # semaphores may only be incremented from one SWDGE queue.  Setting
    # queue_num = lane % 4 after scheduling makes queue<->sem consistency
    # hold by construction (and still spreads work over all 4 queues).
    from concourse.tile_scheduler import PROC_NAME_TO_IDX

    dmasw_lane = {PROC_NAME_TO_IDX[f"DMASW{i}"]: i for i in range(8)}
    gathers = []          # (inst, lane, expected_rows)
    lane_rows = [0.0] * 8
    lane_forced0 = set()  # lanes holding non-gather pool DMAs (queue 0)
    for blk in nc.m.functions[0].blocks:
        for inst in blk.instructions:
            lane = dmasw_lane.get(getattr(inst, "bass_scheduled_proc", None))
            if lane is None:
                continue
            if inst.__class__.__name__ == "InstDMAGatherAnt":
                rows = EXPECT_ROWS.get(inst.num_idxs, inst.num_idxs)
                gathers.append((inst, lane))
                lane_rows[lane] += rows
            else:
                lane_forced0.add(lane)
    assert gathers
    # choose lane->queue (2 lanes per queue) minimizing max queue load;
    # lanes carrying non-gather pool DMAs must map to queue 0
    import itertools

    best = None
    for perm in itertools.permutations(range(8)):
        if any(perm.index(l) // 2 != 0 for l in lane_forced0):
            continue
        loads = [
            lane_rows[perm[2 * q]] + lane_rows[perm[2 * q + 1]]
            for q in range(4)
        ]
        key = (max(loads), -min(loads))
        if best is None or key < best[0]:
            best = (key, perm)
    lane_q = {}
    for q in range(4):
        lane_q[best[1][2 * q]] = q
        lane_q[best[1][2 * q + 1]] = q
    for inst, lane in gathers:
        inst.queue_num = lane_q[lane]

    nc.compile()
    return nc


def _get_program(bsh, vocab, k, e, sg_tiles, caps, ncores):
    key = (bsh, vocab, k, e, sg_tiles, str(caps), ncores)
    if key not in _prog_cache:
        _prog_cache[key] = _build_program(
            bsh, vocab, k, e, sg_tiles, caps, ncores
        )
    return _prog_cache[key]


def _wrap16(vals, ncols):
    """int16 list -> [128, ncols] tile data: value i at [i%16, i//16],
    replicated across the 8 16-partition groups."""
    assert vals.shape[0] == ncols * 16
    arr = np.ascontiguousarray(vals.reshape(ncols, 16).T)
    return np.tile(arr, (8, 1))


def _host_prep(
    ctx_words, target_words, neg_words, V_emb, U_emb, mask_v, mask_u, mask_neg,
    ncores, sg_tiles, caps,
):
    bf16 = _bf16()
    b, k = neg_words.shape
    vocab, e = V_emb.shape
    bsh = b // ncores
    slots = 2 + k
    tiles = bsh // P
    nsg = tiles // sg_tiles
    sg_pos = sg_tiles * P * slots
    trows = TROWS
    tcols = trows // P
    l1_cols = trows // 16
    l2_cols = sg_pos // 16
    nbuck = 7

    W = np.concatenate(
        [np.asarray(V_emb, dtype=np.float32), np.asarray(U_emb, dtype=np.float32)],
        axis=0,
    ).astype(bf16)

    ctx = np.clip(np.asarray(ctx_words).reshape(b), 0, vocab - 1).astype(np.int64)
    tgt = np.clip(np.asarray(target_words).reshape(b), 0, vocab - 1).astype(np.int64)
    neg = np.clip(np.asarray(neg_words).reshape(b, k), 0, vocab - 1).astype(np.int64)

    # gather ids per position: ids[b_row, slot]
    ids_all = np.empty((b, slots), dtype=np.int32)
    ids_all[:, 0] = ctx
    ids_all[:, 1] = vocab + tgt
    ids_all[:, 2:] = vocab + neg

    # fold mask_v into mask_u / mask_neg (slot 0 stays raw on device)
    mv = np.asarray(mask_v, dtype=np.float32).reshape(b, 1, e)
    import ml_dtypes

    f8 = np.dtype(ml_dtypes.float8_e4m3)
    mu_f = (
        np.asarray(mask_u, dtype=np.float32).reshape(b, 1, e) * mv
    ).reshape(b, e).astype(f8)
    mn_f = (
        np.asarray(mask_neg, dtype=np.float32).reshape(b, k, e) * mv
    ).reshape(b, k * e).astype(f8)

    sub_caps = np.asarray([c for _, c in SUBBUCKETS], dtype=np.int64)
    sub_base = np.concatenate([[0], np.cumsum(sub_caps)])  # staging row base
    # sub-bucket index ranges per bucket
    buck_subs = [
        [si for si, (bb, _) in enumerate(SUBBUCKETS) if bb == bi]
        for bi in range(nbuck)
    ]
    buck_cap = np.asarray(
        [sum(int(sub_caps[si]) for si in buck_subs[bi]) for bi in range(nbuck)]
    )

    in_maps = []
    for c in range(ncores):
        lo = c * bsh
        l1_list = np.empty((nsg, trows), dtype=np.int16)
        l2_list = np.empty((nsg, sg_pos), dtype=np.int16)
        cnt_list = np.empty((nsg, NCALLS), dtype=np.int32)
        for sg in range(nsg):
            rlo = lo + sg * sg_tiles * P
            # position i = col*128 + p ; col = t_in_sg*slots + s
            idsl = ids_all[rlo : rlo + sg_tiles * P]          # [(t p), s]
            ids_pos = (
                idsl.reshape(sg_tiles, P, slots)
                .transpose(0, 2, 1)
                .reshape(-1)
            )  # index by (t, s, p) = position order
            order = np.argsort(ids_pos, kind="stable")
            sids = ids_pos[order]
            bucket = sids >> 15  # // 32768
            counts = np.bincount(bucket, minlength=nbuck)
            if np.any(counts > buck_cap):
                raise RuntimeError(
                    f"bucket overflow: counts={counts} caps={buck_cap}"
                )
            # L1 idx lists per sub-bucket: compact, -1 padded (ucode trims)
            l1 = np.full(trows, -1, dtype=np.int16)
            # staging rank (absolute gather position) per sorted element
            rank = np.empty(sg_pos, dtype=np.int64)
            off = 0
            for bi in range(nbuck):
                cnt_b = int(counts[bi])
                seg = sids[off : off + cnt_b] - bi * BUCKET
                used = 0
                for si in buck_subs[bi]:
                    take = min(cnt_b - used, int(sub_caps[si]))
                    sb = sub_base[si]
                    l1[sb : sb + take] = seg[used : used + take].astype(np.int16)
                    rank[off + used : off + used + take] = sb + np.arange(take)
                    cnt_list[sg, si] = take
                    used += take
                off += cnt_b
            # staging row of gather position j is (j%128)*tcols + j//128
            srow = (rank % P) * tcols + (rank // P)
            l2 = np.empty(sg_pos, dtype=np.int16)
            l2[order] = srow.astype(np.int16)
            l1_list[sg] = l1
            l2_list[sg] = l2
        l1m = np.concatenate(
            [_wrap16(l1_list[sg], l1_cols) for sg in range(nsg)], axis=1
        )
        l2m = np.concatenate(
            [_wrap16(l2_list[sg], l2_cols) for sg in range(nsg)], axis=1
        )
        cntm = np.tile(cnt_list.reshape(1, nsg * NCALLS), (P, 1))
        in_maps.append(
            {
                "w": W,
                "l1i": l1m,
                "l2i": l2m,
                "cnt": cntm,
                "mu": mu_f[lo : lo + bsh],
                "mn": mn_f[lo : lo + bsh],
            }
        )
    return in_maps


def kernel(
    ctx_words, target_words, neg_words, V_emb, U_emb, mask_v, mask_u, mask_neg
):
    from concourse.bass_utils import run_bass_kernel_spmd

    b, k = neg_words.shape
    vocab, e = V_emb.shape
    bsh = b // NCORES

    nc = _get_program(bsh, vocab, k, e, SG_TILES, CAPS, NCORES)
    in_maps = _host_prep(
        ctx_words, target_words, neg_words, V_emb, U_emb,
        mask_v, mask_u, mask_neg, NCORES, SG_TILES, CAPS,
    )
    res = run_bass_kernel_spmd(nc, in_maps, core_ids=list(range(NCORES)))
    total = np.float64(0.0)
    for c in range(NCORES):
        total += np.float64(
            res.results[c]["out"].astype(np.float64).sum()
        )
    return np.float32(total / b)
